# revision 9
# baseline (speedup 1.0000x reference)
"""TRN2 Bass kernel for nn_Encoder (two-phase LSTM over huge batch).

Self-contained: takes the FULL unsharded inputs, shards the batch across
8 NeuronCores (pure data parallel), runs a Bass/Tile kernel per core via
run_bass_kernel_spmd, and reassembles the full outputs.

Device layout (per core, batch B_c = 65536):
  - batch split into 8 passes of 16*512; slice s=0..15 covers 512 columns
    of a pass; SBUF partition p = 8*s + r  <->  (slice s, feature r).
  - one fp16 matmul per gate bank per step: M=128, K=128, block-diagonal
    lhsT (16 8x8 blocks) reads the whole h/x tile in place and produces
    that bank for all 16 slices at once.
  - x-tiles pack 3 timesteps (row 2*tau+k = x[t0+tau][k]) plus a ones row
    that carries the fused bias; the host bakes this layout (fp16) so
    every DMA is a contiguous [128, 512] transfer.
  - PSUM gate banks [F, I, O, G] as one [128, 4, 512] tile from a rotating
    2-slot pool.
  - ACT is the bottleneck engine (~40 transcendentals per element per
    step at 1 elem/cycle/lane + ~352 cy fixed cost per ACTIVATE), so the
    schedule minimizes ACT instructions:
      * G-gate weights/bias are pre-doubled on the host, so
        tanh(g) = 2*sigmoid(2g) - 1 and ALL FOUR banks go through a
        single Sigmoid ACTIVATE per step (PSUM src, 2048 elems).
      * tanh(c) is batched across a 4-chain group into one ACTIVATE
        (chains keep c in one contiguous group tile).
      * group tanh instructions are emitted offset by one chain so ACT
        never waits on the DVE cell-update chain.
  - DVE (all fp16): tg=2*S_G-1 (tensor_scalar, 4x mode), u=F*c, v=I*tg,
    c=u+v, h=O*tanh_c (tensor_tensor, 2x mode).
  - input embedding + biases are folded into the lhsT weights on the host
    (gates = x @ (W_ih W_in).T + h @ W_hh.T + (W_ih b_in + b_ih + b_hh)).
"""

import os
import sys

for _p in ("/opt/trn_rl_repo", "/root/.axon_site/_ro/trn_rl_repo"):
    if os.path.isdir(_p) and _p not in sys.path:
        sys.path.insert(0, _p)
        break

import numpy as np

import concourse.bacc as bacc
import concourse.mybir as mybir
import concourse.tile as tile
from concourse import bass_utils

F32 = mybir.dt.float32
F16 = mybir.dt.float16
AF = mybir.ActivationFunctionType
ALU = mybir.AluOpType

B = 524288
N_CORES = 8
B_C = B // N_CORES
N = 512
SLICES = 16
PASS = SLICES * N
N_PASS = B_C // PASS
T_OBS, T_PRE, IN, H = 8, 12, 2, 8
XPACK = 3
N_CHUNK_OBS = (T_OBS + XPACK - 1) // XPACK
N_CHUNK_PRE = (T_PRE + XPACK - 1) // XPACK
N_CHAINS = 8
GRP = 4  # chains per tanh(c) batch group
# bank order: F, I, O, G (sigmoid banks contiguous, tanh last); pytorch
# gate order in the weight rows is i, f, g, o.
BANK_GATE = [1, 0, 3, 2]


# ---------------------------------------------------------------- host prep

def _make_weights(W_in, b_in, W_ih, W_hh, b_ih, b_hh):
    """lhsT arrays: w_gx [XPACK, 128, 4, 128] (tau,p,bank,m), w_gh [128,4,128].

    Block-diagonal over the 16 slices: one M=128, K=128 matmul per gate bank
    computes that bank for all 16 slices at once.  Bank 3 (the candidate
    gate g) is pre-scaled by 2 so tanh(g) = 2*sigmoid(2g) - 1 on device.
    """
    Wx = (W_ih @ W_in).astype(np.float32)
    bias = (W_ih @ b_in + b_ih + b_hh).astype(np.float32)
    w_gx = np.zeros((XPACK, 128, 4, 128), np.float32)
    w_gh = np.zeros((128, 4, 128), np.float32)
    for b in range(4):
        g = BANK_GATE[b]
        sc = 2.0 if b == 3 else 1.0
        for s in range(16):
            for r in range(H):
                col = 8 * s + r
                for tau in range(XPACK):
                    for k in range(IN):
                        w_gx[tau, 8 * s + 2 * tau + k, b, col] = sc * Wx[g * H + r, k]
                    w_gx[tau, 8 * s + 6, b, col] = sc * bias[g * H + r]
                w_gh[8 * s: 8 * s + H, b, col] = sc * W_hh[g * H + r, :]
    return w_gx.astype(np.float16), w_gh.astype(np.float16)


def _shuffle_state(aT):
    """[8, B_c] -> [128, N_PASS, N] device layout (8s+r, pass, n)."""
    return np.ascontiguousarray(
        aT.reshape(H, N_PASS, SLICES, N).transpose(2, 0, 1, 3).reshape(
            128, N_PASS, N).astype(np.float16))


def _unshuffle_state(dev):
    """[128, N_PASS, N] -> [8, B_c]."""
    return dev.reshape(SLICES, H, N_PASS, N).transpose(1, 2, 0, 3).reshape(
        H, B_C)


def _pack_x(x):
    """[T, 2, B_c] -> [n_chunk, N_PASS, 128, N]: 3 steps + ones row baked."""
    T = x.shape[0]
    n_chunk = (T + XPACK - 1) // XPACK
    out = np.zeros((n_chunk, N_PASS, SLICES, 8, N), np.float32)
    out[:, :, :, 6, :] = 1.0
    for tau in range(XPACK):
        for k in range(IN):
            for t3 in range(n_chunk):
                t = t3 * XPACK + tau
                if t < T:
                    out[t3, :, :, 2 * tau + k, :] = x[t, k].reshape(
                        N_PASS, SLICES, N)
    return np.ascontiguousarray(
        out.transpose(0, 2, 3, 1, 4).reshape(n_chunk, 128, N_PASS, N).astype(
            np.float16))


def _prep_core_inputs(inputs, lo, hi, weights):
    g = lambda k: np.asarray(inputs[k], np.float32)
    d = {}
    d["x_obs"] = _pack_x(
        np.ascontiguousarray(g("obs_traj_rel")[:, lo:hi, :].transpose(0, 2, 1)))
    d["x_pre"] = _pack_x(
        np.ascontiguousarray(g("pre_traj_rel")[:, lo:hi, :].transpose(0, 2, 1)))
    d["hT0"] = _shuffle_state(np.ascontiguousarray(g("h0")[lo:hi].T))
    d["cT0"] = _shuffle_state(np.ascontiguousarray(g("c0")[lo:hi].T))
    d["cT0_pre"] = _shuffle_state(np.ascontiguousarray(g("c0_pre")[lo:hi].T))
    d.update(weights)
    return d


# ------------------------------------------------------------- device build

def _build_kernel(tc, outs, ins):
    nc = tc.nc
    state = tc.alloc_tile_pool(name="state", bufs=1)
    psump = tc.alloc_tile_pool(name="psum", bufs=2, space="PSUM")

    wsb = {}
    for key in ("w_gx_obs", "w_gx_pre"):
        wsb[key] = state.tile([128, XPACK, 4, 128], F16, name=key + "_sb",
                              tag=key)
    for key in ("w_gh_obs", "w_gh_pre"):
        wsb[key] = state.tile([128, 4, 128], F16, name=key + "_sb", tag=key)

    def dma_weights(phase):
        for tau in range(XPACK):
            nc.sync.dma_start(wsb[f"w_gx_{phase}"][:, tau],
                              ins[f"w_gx_{phase}"][tau])
        nc.sync.dma_start(wsb[f"w_gh_{phase}"], ins[f"w_gh_{phase}"])

    # group tiles (4 chains each): h, c, tanh(c); x double-buffered for all 8
    n_grp = N_CHAINS // GRP
    hgrp = [state.tile([128, GRP, N], F16, name=f"hg_{g}", tag=f"hg_{g}")
            for g in range(n_grp)]
    cgrp = [state.tile([128, GRP, N], F16, name=f"cg_{g}", tag=f"cg_{g}")
            for g in range(n_grp)]
    tgrp = [state.tile([128, GRP, N], F16, name=f"tg_{g}", tag=f"tg_{g}")
            for g in range(n_grp)]
    xall = [state.tile([128, N_CHAINS, N], F16, name=f"xa_{i}", tag=f"xa_{i}")
            for i in range(2)]

    chains = []
    for ci in range(N_CHAINS):
        ch = {}
        for nm in ("gbar", "u", "v"):
            ch[nm] = state.tile([128, N], F16, name=f"{nm}_{ci}",
                                tag=f"{nm}_{ci}")
        ch["T"] = [
            state.tile([128, 4, N], F16, name=f"T_{ci}_{pb}", tag=f"T_{ci}_{pb}")
            for pb in range(2)
        ]
        ch["h"] = hgrp[ci // GRP][:, ci % GRP, :]
        ch["c"] = cgrp[ci // GRP][:, ci % GRP, :]
        ch["tc"] = tgrp[ci // GRP][:, ci % GRP, :]
        chains.append(ch)

    # PE warm-up: dummy matmuls keep the PE HAM busy during initial DMAs
    warm = state.tile([128, 128], F16, name="warm", tag="warm")
    nc.vector.memset(warm, 0.0)
    wps = psump.tile([128, 4, 512], F32, name="wps", tag="ps")
    for _ in range(24):
        nc.tensor.matmul(wps[:, 0, :128], warm, warm, start=True, stop=True)

    # global x-chunk schedule: (which, chunk idx, first step)
    chunk_seq = [("obs", k, k * XPACK) for k in range(N_CHUNK_OBS)] + \
                [("pre", k, T_OBS + k * XPACK) for k in range(N_CHUNK_PRE)]
    start_to_chunk = {st: i for i, (_, _, st) in enumerate(chunk_seq)}

    def dma_chunk(gi):
        which, k, _ = chunk_seq[gi]
        nc.sync.dma_start(xall[gi % 2], ins[f"x_{which}"][k])

    def dma_chunk_half(gi, g):
        which, k, _ = chunk_seq[gi]
        sl = slice(g * GRP, (g + 1) * GRP)
        nc.sync.dma_start(xall[gi % 2][:, sl, :],
                          ins[f"x_{which}"][k][:, sl, :])

    def mm_block(ch, wgx, wgh, tau, xt, pb):
        ps = psump.tile([128, 4, 512], F32, name="ps", tag="ps")
        for b in range(4):
            out = ps[:, b, :N]
            nc.tensor.matmul(out, wgx[:, tau, b, :], xt,
                             start=True, stop=False)
            nc.tensor.matmul(out, wgh[:, b, :], ch["h"],
                             start=False, stop=True)
        # single Sigmoid over all 4 banks (G pre-scaled by 2 in weights)
        T = ch["T"][pb]
        nc.scalar.activation(T[:, :, :], ps[:, :, :], AF.Sigmoid)
        # tg = 2*S_G - 1 == tanh(g)   (tensor_scalar, 4x mode)
        nc.vector.tensor_scalar(ch["gbar"], T[:, 3, :], 2.0, -1.0,
                                ALU.mult, ALU.add)
        nc.vector.tensor_mul(ch["u"], T[:, 0, :], ch["c"])   # F*c
        nc.vector.tensor_mul(ch["v"], T[:, 1, :], ch["gbar"])  # I*tanh(g)
        nc.vector.tensor_add(ch["c"], ch["u"], ch["v"])      # c' = u+v

    def group_tanh(g, pb):
        # one ACTIVATE for the whole group's c, then h = O * tanh(c)
        nc.scalar.activation(tgrp[g][:, :, :], cgrp[g][:, :, :], AF.Tanh)
        for ci in range(g * GRP, (g + 1) * GRP):
            ch = chains[ci]
            nc.vector.tensor_mul(ch["h"], ch["T"][pb][:, 2, :], ch["tc"])

    def grp_slice(ap, g):
        return ap[:, g * GRP:(g + 1) * GRP, :]

    assert N_PASS == N_CHAINS
    for t in range(T_OBS + T_PRE):
        if t < T_OBS:
            which, tt = "obs", t
        else:
            which, tt = "pre", t - T_OBS
        wgx, wgh = wsb[f"w_gx_{which}"], wsb[f"w_gh_{which}"]
        t3, tau = divmod(tt, XPACK)
        gi = start_to_chunk[t - tau]
        for ci in range(N_CHAINS):
            if ci == 1 and t > 0:
                group_tanh(1, (t - 1) % 2)  # tanh(c), chains 4-7, step t-1
                if t == T_OBS:
                    # h after obs phase (chains 4-7) + cell re-init
                    nc.sync.dma_start(grp_slice(outs["hT_obs"], 1), hgrp[1])
                    nc.sync.dma_start(cgrp[1], grp_slice(ins["cT0_pre"], 1))
            if ci == 0:
                if t == 0:
                    which0 = chunk_seq[0][0]
                    # critical-path first: chain-0 x/h, tau-0 weights
                    nc.sync.dma_start(xall[0][:, 0, :],
                                      ins[f"x_{which0}"][0][:, 0, :])
                    nc.sync.dma_start(hgrp[0][:, 0, :], ins["hT0"][:, 0, :])
                    nc.sync.dma_start(wsb["w_gx_obs"][:, 0],
                                      ins["w_gx_obs"][0])
                    nc.sync.dma_start(wsb["w_gh_obs"], ins["w_gh_obs"])
                    for j in range(1, GRP):
                        nc.sync.dma_start(xall[0][:, j, :],
                                          ins[f"x_{which0}"][0][:, j, :])
                        nc.sync.dma_start(hgrp[0][:, j, :],
                                          ins["hT0"][:, j, :])
                    nc.sync.dma_start(cgrp[0], grp_slice(ins["cT0"], 0))
                    dma_chunk_half(0, 1)
                    nc.sync.dma_start(hgrp[1], grp_slice(ins["hT0"], 1))
                    nc.sync.dma_start(cgrp[1], grp_slice(ins["cT0"], 1))
                    for tau in range(1, XPACK):
                        nc.sync.dma_start(wsb["w_gx_obs"][:, tau],
                                          ins["w_gx_obs"][tau])
                if t == 1:
                    dma_weights("pre")
                    dma_chunk(1)  # prefetch chunk 1 (needed at t=3)
                if t == T_OBS:
                    nc.sync.dma_start(grp_slice(outs["hT_obs"], 0), hgrp[0])
                    nc.sync.dma_start(cgrp[0], grp_slice(ins["cT0_pre"], 0))
                if tau == 0 and t > 1 and gi + 1 < len(chunk_seq):
                    dma_chunk(gi + 1)  # prefetch next chunk
            mm_block(chains[ci], wgx, wgh, tau, xall[gi % 2][:, ci, :],
                     t % 2)
            if ci == GRP:
                group_tanh(0, t % 2)  # tanh(c) for chains 0-3 of step t
                if t == T_OBS + T_PRE - 1:
                    nc.sync.dma_start(grp_slice(outs["hT_pre"], 0), hgrp[0])
    group_tanh(1, (T_OBS + T_PRE - 1) % 2)  # last step, chains 4-7
    nc.sync.dma_start(grp_slice(outs["hT_pre"], 1), hgrp[1])

    state.release()
    psump.release()


_CACHED = {}


def _get_program():
    if "nc" in _CACHED:
        return _CACHED["nc"], _CACHED["names"]
    nc = bacc.Bacc("TRN2", target_bir_lowering=False, debug=False,
                   enable_asserts=False, num_devices=N_CORES)
    in_shapes = {
        "x_obs": (N_CHUNK_OBS, 128, N_PASS, N),
        "x_pre": (N_CHUNK_PRE, 128, N_PASS, N),
        "hT0": (128, N_PASS, N),
        "cT0": (128, N_PASS, N),
        "cT0_pre": (128, N_PASS, N),
        "w_gx_obs": (XPACK, 128, 4, 128),
        "w_gh_obs": (128, 4, 128),
        "w_gx_pre": (XPACK, 128, 4, 128),
        "w_gh_pre": (128, 4, 128),
    }
    ins = {
        k: nc.dram_tensor(k, list(s), F16, kind="ExternalInput").ap()
        for k, s in in_shapes.items()
    }
    outs = {
        k: nc.dram_tensor(k, [128, N_PASS, N], F16, kind="ExternalOutput").ap()
        for k in ("hT_obs", "hT_pre")
    }
    with tile.TileContext(nc) as tc:
        _build_kernel(tc, outs, ins)
    nc.compile()
    _CACHED["nc"] = nc
    _CACHED["names"] = list(in_shapes)
    return nc, _CACHED["names"]


def run(inputs, trace=False, trace_kwargs=None):
    """Run the kernel on 8 cores; returns ((c_out, x_out), BassKernelResults)."""
    nc, _ = _get_program()
    g = lambda k: np.asarray(inputs[k], np.float32)
    wgx_o, wgh_o = _make_weights(g("W_in"), g("b_in"), g("W_ih_obs"),
                                 g("W_hh_obs"), g("b_ih_obs"), g("b_hh_obs"))
    wgx_p, wgh_p = _make_weights(g("W_in"), g("b_in"), g("W_ih_pre"),
                                 g("W_hh_pre"), g("b_ih_pre"), g("b_hh_pre"))
    weights = {"w_gx_obs": wgx_o, "w_gh_obs": wgh_o,
               "w_gx_pre": wgx_p, "w_gh_pre": wgh_p}
    in_maps = [
        _prep_core_inputs(inputs, c * B_C, (c + 1) * B_C, weights)
        for c in range(N_CORES)
    ]
    res = bass_utils.run_bass_kernel_spmd(
        nc, in_maps, core_ids=list(range(N_CORES)), trace=trace,
        **(trace_kwargs or {}))
    hT_obs = np.concatenate(
        [_unshuffle_state(res.results[c]["hT_obs"]) for c in range(N_CORES)],
        axis=1)
    hT_pre = np.concatenate(
        [_unshuffle_state(res.results[c]["hT_pre"]) for c in range(N_CORES)],
        axis=1)
    c_out = hT_obs.reshape(B, H).astype(np.float32)
    x_out = hT_pre.reshape(B, H).astype(np.float32)
    return (c_out, x_out), res


def kernel(**inputs):
    (c_out, x_out), _ = run(inputs)
    return c_out, x_out


# revision 10
# speedup vs baseline: 1.0294x; 1.0294x over previous
"""TRN2 Bass kernel for nn_Encoder (two-phase LSTM over huge batch).

Self-contained: takes the FULL unsharded inputs, shards the batch across
8 NeuronCores (pure data parallel), runs a Bass/Tile kernel per core via
run_bass_kernel_spmd, and reassembles the full outputs.

Device layout (per core, batch B_c = 65536):
  - batch split into 8 passes of 16*512; slice s=0..15 covers 512 columns
    of a pass; SBUF partition p = 8*s + r  <->  (slice s, feature r).
  - one fp16 matmul per gate bank per step: M=128, K=128, block-diagonal
    lhsT (16 8x8 blocks) reads the whole h/x tile in place and produces
    that bank for all 16 slices at once.
  - x-tiles pack 3 timesteps (row 2*tau+k = x[t0+tau][k]) plus a ones row
    that carries the fused bias; the host bakes this layout (fp16) so
    every DMA is a contiguous [128, 512] transfer.
  - PSUM gate banks [F, I, O, G] as one [128, 4, 512] tile from a rotating
    2-slot pool.
  - ACT is the bottleneck engine (~40 transcendentals per element per
    step at 1 elem/cycle/lane + ~352 cy fixed cost per ACTIVATE), so the
    schedule minimizes ACT instructions:
      * G-gate weights/bias are pre-doubled on the host, so
        tanh(g) = 2*sigmoid(2g) - 1 and ALL FOUR banks go through a
        single Sigmoid ACTIVATE per step (PSUM src, 2048 elems).
      * tanh(c) is batched across a 4-chain group into one ACTIVATE
        (chains keep c in one contiguous group tile).
      * group tanh instructions are emitted offset by one chain so ACT
        never waits on the DVE cell-update chain.
  - DVE (all fp16): tg=2*S_G-1 (tensor_scalar, 4x mode), u=F*c, v=I*tg,
    c=u+v, h=O*tanh_c (tensor_tensor, 2x mode).
  - input embedding + biases are folded into the lhsT weights on the host
    (gates = x @ (W_ih W_in).T + h @ W_hh.T + (W_ih b_in + b_ih + b_hh)).
"""

import os
import sys

for _p in ("/opt/trn_rl_repo", "/root/.axon_site/_ro/trn_rl_repo"):
    if os.path.isdir(_p) and _p not in sys.path:
        sys.path.insert(0, _p)
        break

import numpy as np

import concourse.bacc as bacc
import concourse.mybir as mybir
import concourse.tile as tile
from concourse import bass_utils

F32 = mybir.dt.float32
F16 = mybir.dt.float16
AF = mybir.ActivationFunctionType
ALU = mybir.AluOpType

B = 524288
N_CORES = 8
B_C = B // N_CORES
N = 512
SLICES = 16
PASS = SLICES * N
N_PASS = B_C // PASS
T_OBS, T_PRE, IN, H = 8, 12, 2, 8
XPACK = 3
N_CHUNK_OBS = (T_OBS + XPACK - 1) // XPACK
N_CHUNK_PRE = (T_PRE + XPACK - 1) // XPACK
N_CHAINS = 8
GRP = 4  # chains per tanh(c) batch group
# bank order: F, I, O, G (sigmoid banks contiguous, tanh last); pytorch
# gate order in the weight rows is i, f, g, o.
BANK_GATE = [1, 0, 3, 2]


# ---------------------------------------------------------------- host prep

def _make_weights(W_in, b_in, W_ih, W_hh, b_ih, b_hh):
    """lhsT arrays: w_gx [XPACK, 128, 4, 128] (tau,p,bank,m), w_gh [128,4,128].

    Block-diagonal over the 16 slices: one M=128, K=128 matmul per gate bank
    computes that bank for all 16 slices at once.  Bank 3 (the candidate
    gate g) is pre-scaled by 2 so tanh(g) = 2*sigmoid(2g) - 1 on device.
    """
    Wx = (W_ih @ W_in).astype(np.float32)
    bias = (W_ih @ b_in + b_ih + b_hh).astype(np.float32)
    w_gx = np.zeros((XPACK, 128, 4, 128), np.float32)
    w_gh = np.zeros((128, 4, 128), np.float32)
    for b in range(4):
        g = BANK_GATE[b]
        sc = 2.0 if b == 3 else 1.0
        for s in range(16):
            for r in range(H):
                col = 8 * s + r
                for tau in range(XPACK):
                    for k in range(IN):
                        w_gx[tau, 8 * s + 2 * tau + k, b, col] = sc * Wx[g * H + r, k]
                    w_gx[tau, 8 * s + 6, b, col] = sc * bias[g * H + r]
                w_gh[8 * s: 8 * s + H, b, col] = sc * W_hh[g * H + r, :]
    return w_gx.astype(np.float16), w_gh.astype(np.float16)


def _shuffle_state(aT):
    """[8, B_c] -> [128, N_PASS, N] device layout (8s+r, pass, n)."""
    return np.ascontiguousarray(
        aT.reshape(H, N_PASS, SLICES, N).transpose(2, 0, 1, 3).reshape(
            128, N_PASS, N).astype(np.float16))


def _unshuffle_state(dev):
    """[128, N_PASS, N] -> [8, B_c]."""
    return dev.reshape(SLICES, H, N_PASS, N).transpose(1, 2, 0, 3).reshape(
        H, B_C)


def _pack_x(x):
    """[T, 2, B_c] -> [n_chunk, N_PASS, 128, N]: 3 steps + ones row baked."""
    T = x.shape[0]
    n_chunk = (T + XPACK - 1) // XPACK
    out = np.zeros((n_chunk, N_PASS, SLICES, 8, N), np.float32)
    out[:, :, :, 6, :] = 1.0
    for tau in range(XPACK):
        for k in range(IN):
            for t3 in range(n_chunk):
                t = t3 * XPACK + tau
                if t < T:
                    out[t3, :, :, 2 * tau + k, :] = x[t, k].reshape(
                        N_PASS, SLICES, N)
    return np.ascontiguousarray(
        out.transpose(0, 2, 3, 1, 4).reshape(n_chunk, 128, N_PASS, N).astype(
            np.float16))


def _prep_core_inputs(inputs, lo, hi, weights):
    g = lambda k: np.asarray(inputs[k], np.float32)
    d = {}
    d["x_obs"] = _pack_x(
        np.ascontiguousarray(g("obs_traj_rel")[:, lo:hi, :].transpose(0, 2, 1)))
    d["x_pre"] = _pack_x(
        np.ascontiguousarray(g("pre_traj_rel")[:, lo:hi, :].transpose(0, 2, 1)))
    d["hT0"] = _shuffle_state(np.ascontiguousarray(g("h0")[lo:hi].T))
    d["cT0"] = _shuffle_state(np.ascontiguousarray(g("c0")[lo:hi].T))
    d["cT0_pre"] = _shuffle_state(np.ascontiguousarray(g("c0_pre")[lo:hi].T))
    d.update(weights)
    return d


# ------------------------------------------------------------- device build

def _build_kernel(tc, outs, ins):
    nc = tc.nc
    state = tc.alloc_tile_pool(name="state", bufs=1)
    psump = tc.alloc_tile_pool(name="psum", bufs=2, space="PSUM")

    wsb = {}
    for key in ("w_gx_obs", "w_gx_pre"):
        wsb[key] = state.tile([128, XPACK, 4, 128], F16, name=key + "_sb",
                              tag=key)
    for key in ("w_gh_obs", "w_gh_pre"):
        wsb[key] = state.tile([128, 4, 128], F16, name=key + "_sb", tag=key)

    def dma_weights(phase):
        nc.sync.dma_start(wsb[f"w_gx_{phase}"],
                          ins[f"w_gx_{phase}"].rearrange("t p b m -> p t b m"))
        nc.sync.dma_start(wsb[f"w_gh_{phase}"], ins[f"w_gh_{phase}"])

    # group tiles (4 chains each): h, c, tanh(c); x double-buffered for all 8
    n_grp = N_CHAINS // GRP
    hgrp = [state.tile([128, GRP, N], F16, name=f"hg_{g}", tag=f"hg_{g}")
            for g in range(n_grp)]
    cgrp = [state.tile([128, GRP, N], F16, name=f"cg_{g}", tag=f"cg_{g}")
            for g in range(n_grp)]
    tgrp = [state.tile([128, GRP, N], F16, name=f"tg_{g}", tag=f"tg_{g}")
            for g in range(n_grp)]
    xall = [state.tile([128, N_CHAINS, N], F16, name=f"xa_{i}", tag=f"xa_{i}")
            for i in range(2)]

    chains = []
    for ci in range(N_CHAINS):
        ch = {}
        for nm in ("gbar", "u", "v"):
            ch[nm] = state.tile([128, N], F16, name=f"{nm}_{ci}",
                                tag=f"{nm}_{ci}")
        ch["T"] = [
            state.tile([128, 4, N], F16, name=f"T_{ci}_{pb}", tag=f"T_{ci}_{pb}")
            for pb in range(2)
        ]
        ch["h"] = hgrp[ci // GRP][:, ci % GRP, :]
        ch["c"] = cgrp[ci // GRP][:, ci % GRP, :]
        ch["tc"] = tgrp[ci // GRP][:, ci % GRP, :]
        chains.append(ch)

    # PE warm-up: dummy matmuls keep the PE HAM busy during initial DMAs
    warm = state.tile([128, 128], F16, name="warm", tag="warm")
    nc.vector.memset(warm, 0.0)
    wps = psump.tile([128, 4, 512], F32, name="wps", tag="ps")
    for _ in range(17):
        nc.tensor.matmul(wps[:, 0, :128], warm, warm, start=True, stop=True)

    # global x-chunk schedule: (which, chunk idx, first step)
    chunk_seq = [("obs", k, k * XPACK) for k in range(N_CHUNK_OBS)] + \
                [("pre", k, T_OBS + k * XPACK) for k in range(N_CHUNK_PRE)]
    start_to_chunk = {st: i for i, (_, _, st) in enumerate(chunk_seq)}

    def dma_chunk(gi):
        which, k, _ = chunk_seq[gi]
        nc.sync.dma_start(xall[gi % 2], ins[f"x_{which}"][k])

    def dma_chunk_half(gi, g):
        which, k, _ = chunk_seq[gi]
        sl = slice(g * GRP, (g + 1) * GRP)
        nc.sync.dma_start(xall[gi % 2][:, sl, :],
                          ins[f"x_{which}"][k][:, sl, :])

    def mm_block(ch, wgx, wgh, tau, xt, pb):
        ps = psump.tile([128, 4, 512], F32, name="ps", tag="ps")
        for b in range(4):
            out = ps[:, b, :N]
            nc.tensor.matmul(out, wgx[:, tau, b, :], xt,
                             start=True, stop=False)
            nc.tensor.matmul(out, wgh[:, b, :], ch["h"],
                             start=False, stop=True)
        # single Sigmoid over all 4 banks (G pre-scaled by 2 in weights)
        T = ch["T"][pb]
        nc.scalar.activation(T[:, :, :], ps[:, :, :], AF.Sigmoid)
        # tg = 2*S_G - 1 == tanh(g)   (tensor_scalar, 4x mode)
        nc.vector.tensor_scalar(ch["gbar"], T[:, 3, :], 2.0, -1.0,
                                ALU.mult, ALU.add)
        nc.vector.tensor_mul(ch["u"], T[:, 0, :], ch["c"])   # F*c
        nc.vector.tensor_mul(ch["v"], T[:, 1, :], ch["gbar"])  # I*tanh(g)
        nc.vector.tensor_add(ch["c"], ch["u"], ch["v"])      # c' = u+v

    def group_tanh(g, pb):
        # one ACTIVATE for the whole group's c, then h = O * tanh(c)
        nc.scalar.activation(tgrp[g][:, :, :], cgrp[g][:, :, :], AF.Tanh)
        for ci in range(g * GRP, (g + 1) * GRP):
            ch = chains[ci]
            nc.vector.tensor_mul(ch["h"], ch["T"][pb][:, 2, :], ch["tc"])

    def grp_slice(ap, g):
        return ap[:, g * GRP:(g + 1) * GRP, :]

    assert N_PASS == N_CHAINS
    for t in range(T_OBS + T_PRE):
        if t < T_OBS:
            which, tt = "obs", t
        else:
            which, tt = "pre", t - T_OBS
        wgx, wgh = wsb[f"w_gx_{which}"], wsb[f"w_gh_{which}"]
        t3, tau = divmod(tt, XPACK)
        gi = start_to_chunk[t - tau]
        for ci in range(N_CHAINS):
            if ci == 1 and t > 0:
                group_tanh(1, (t - 1) % 2)  # tanh(c), chains 4-7, step t-1
                if t == T_OBS:
                    # h after obs phase (chains 4-7) + cell re-init
                    nc.sync.dma_start(grp_slice(outs["hT_obs"], 1), hgrp[1])
                    nc.sync.dma_start(cgrp[1], grp_slice(ins["cT0_pre"], 1))
            if ci == 0:
                if t == 0:
                    dma_weights("obs")
                    which0 = chunk_seq[0][0]
                    for j in range(GRP):
                        nc.sync.dma_start(xall[0][:, j, :],
                                          ins[f"x_{which0}"][0][:, j, :])
                        nc.sync.dma_start(hgrp[0][:, j, :],
                                          ins["hT0"][:, j, :])
                    nc.sync.dma_start(cgrp[0], grp_slice(ins["cT0"], 0))
                    dma_chunk_half(0, 1)
                    nc.sync.dma_start(hgrp[1], grp_slice(ins["hT0"], 1))
                    nc.sync.dma_start(cgrp[1], grp_slice(ins["cT0"], 1))
                if t == 1:
                    dma_weights("pre")
                    dma_chunk(1)  # prefetch chunk 1 (needed at t=3)
                if t == T_OBS:
                    nc.sync.dma_start(grp_slice(outs["hT_obs"], 0), hgrp[0])
                    nc.sync.dma_start(cgrp[0], grp_slice(ins["cT0_pre"], 0))
                if tau == 0 and t > 1 and gi + 1 < len(chunk_seq):
                    dma_chunk(gi + 1)  # prefetch next chunk
            mm_block(chains[ci], wgx, wgh, tau, xall[gi % 2][:, ci, :],
                     t % 2)
            if ci == GRP:
                group_tanh(0, t % 2)  # tanh(c) for chains 0-3 of step t
                if t == T_OBS + T_PRE - 1:
                    nc.sync.dma_start(grp_slice(outs["hT_pre"], 0), hgrp[0])
    group_tanh(1, (T_OBS + T_PRE - 1) % 2)  # last step, chains 4-7
    nc.sync.dma_start(grp_slice(outs["hT_pre"], 1), hgrp[1])

    state.release()
    psump.release()


_CACHED = {}


def _get_program():
    if "nc" in _CACHED:
        return _CACHED["nc"], _CACHED["names"]
    nc = bacc.Bacc("TRN2", target_bir_lowering=False, debug=False,
                   enable_asserts=False, num_devices=N_CORES)
    in_shapes = {
        "x_obs": (N_CHUNK_OBS, 128, N_PASS, N),
        "x_pre": (N_CHUNK_PRE, 128, N_PASS, N),
        "hT0": (128, N_PASS, N),
        "cT0": (128, N_PASS, N),
        "cT0_pre": (128, N_PASS, N),
        "w_gx_obs": (XPACK, 128, 4, 128),
        "w_gh_obs": (128, 4, 128),
        "w_gx_pre": (XPACK, 128, 4, 128),
        "w_gh_pre": (128, 4, 128),
    }
    ins = {
        k: nc.dram_tensor(k, list(s), F16, kind="ExternalInput").ap()
        for k, s in in_shapes.items()
    }
    outs = {
        k: nc.dram_tensor(k, [128, N_PASS, N], F16, kind="ExternalOutput").ap()
        for k in ("hT_obs", "hT_pre")
    }
    with tile.TileContext(nc) as tc:
        _build_kernel(tc, outs, ins)
    nc.compile()
    _CACHED["nc"] = nc
    _CACHED["names"] = list(in_shapes)
    return nc, _CACHED["names"]


def run(inputs, trace=False, trace_kwargs=None):
    """Run the kernel on 8 cores; returns ((c_out, x_out), BassKernelResults)."""
    nc, _ = _get_program()
    g = lambda k: np.asarray(inputs[k], np.float32)
    wgx_o, wgh_o = _make_weights(g("W_in"), g("b_in"), g("W_ih_obs"),
                                 g("W_hh_obs"), g("b_ih_obs"), g("b_hh_obs"))
    wgx_p, wgh_p = _make_weights(g("W_in"), g("b_in"), g("W_ih_pre"),
                                 g("W_hh_pre"), g("b_ih_pre"), g("b_hh_pre"))
    weights = {"w_gx_obs": wgx_o, "w_gh_obs": wgh_o,
               "w_gx_pre": wgx_p, "w_gh_pre": wgh_p}
    in_maps = [
        _prep_core_inputs(inputs, c * B_C, (c + 1) * B_C, weights)
        for c in range(N_CORES)
    ]
    res = bass_utils.run_bass_kernel_spmd(
        nc, in_maps, core_ids=list(range(N_CORES)), trace=trace,
        **(trace_kwargs or {}))
    hT_obs = np.concatenate(
        [_unshuffle_state(res.results[c]["hT_obs"]) for c in range(N_CORES)],
        axis=1)
    hT_pre = np.concatenate(
        [_unshuffle_state(res.results[c]["hT_pre"]) for c in range(N_CORES)],
        axis=1)
    c_out = hT_obs.reshape(B, H).astype(np.float32)
    x_out = hT_pre.reshape(B, H).astype(np.float32)
    return (c_out, x_out), res


def kernel(**inputs):
    (c_out, x_out), _ = run(inputs)
    return c_out, x_out


# revision 11
# speedup vs baseline: 1.0318x; 1.0024x over previous
"""TRN2 Bass kernel for nn_Encoder (two-phase LSTM over huge batch).

Self-contained: takes the FULL unsharded inputs, shards the batch across
8 NeuronCores (pure data parallel), runs a Bass/Tile kernel per core via
run_bass_kernel_spmd, and reassembles the full outputs.

Device layout (per core, batch B_c = 65536):
  - batch split into 8 passes of 16*512; slice s=0..15 covers 512 columns
    of a pass; SBUF partition p = 8*s + r  <->  (slice s, feature r).
  - one fp16 matmul per gate bank per step: M=128, K=128, block-diagonal
    lhsT (16 8x8 blocks) reads the whole h/x tile in place and produces
    that bank for all 16 slices at once.
  - x-tiles pack 3 timesteps (row 2*tau+k = x[t0+tau][k]) plus a ones row
    that carries the fused bias; the host bakes this layout (fp16) so
    every DMA is a contiguous [128, 512] transfer.
  - PSUM gate banks [F, I, O, G] as one [128, 4, 512] tile from a rotating
    2-slot pool.
  - ACT is the bottleneck engine (~40 transcendentals per element per
    step at 1 elem/cycle/lane + ~352 cy fixed cost per ACTIVATE), so the
    schedule minimizes ACT instructions:
      * G-gate weights/bias are pre-doubled on the host, so
        tanh(g) = 2*sigmoid(2g) - 1 and ALL FOUR banks go through a
        single Sigmoid ACTIVATE per step (PSUM src, 2048 elems).
      * tanh(c) is batched across a 4-chain group into one ACTIVATE
        (chains keep c in one contiguous group tile).
      * group tanh instructions are emitted offset by one chain so ACT
        never waits on the DVE cell-update chain.
  - DVE (all fp16): tg=2*S_G-1 (tensor_scalar, 4x mode), u=F*c, v=I*tg,
    c=u+v, h=O*tanh_c (tensor_tensor, 2x mode).
  - input embedding + biases are folded into the lhsT weights on the host
    (gates = x @ (W_ih W_in).T + h @ W_hh.T + (W_ih b_in + b_ih + b_hh)).
"""

import os
import sys

for _p in ("/opt/trn_rl_repo", "/root/.axon_site/_ro/trn_rl_repo"):
    if os.path.isdir(_p) and _p not in sys.path:
        sys.path.insert(0, _p)
        break

import numpy as np

import concourse.bacc as bacc
import concourse.mybir as mybir
import concourse.tile as tile
from concourse import bass_utils

F32 = mybir.dt.float32
F16 = mybir.dt.float16
AF = mybir.ActivationFunctionType
ALU = mybir.AluOpType

B = 524288
N_CORES = 8
B_C = B // N_CORES
N = 512
SLICES = 16
PASS = SLICES * N
N_PASS = B_C // PASS
T_OBS, T_PRE, IN, H = 8, 12, 2, 8
XPACK = 3
N_CHUNK_OBS = (T_OBS + XPACK - 1) // XPACK
N_CHUNK_PRE = (T_PRE + XPACK - 1) // XPACK
N_CHAINS = 8
GRP = 4  # chains per tanh(c) batch group
# bank order: F, I, O, G (sigmoid banks contiguous, tanh last); pytorch
# gate order in the weight rows is i, f, g, o.
BANK_GATE = [1, 0, 3, 2]


# ---------------------------------------------------------------- host prep

def _make_weights(W_in, b_in, W_ih, W_hh, b_ih, b_hh):
    """lhsT arrays: w_gx [XPACK, 128, 4, 128] (tau,p,bank,m), w_gh [128,4,128].

    Block-diagonal over the 16 slices: one M=128, K=128 matmul per gate bank
    computes that bank for all 16 slices at once.  Bank 3 (the candidate
    gate g) is pre-scaled by 2 so tanh(g) = 2*sigmoid(2g) - 1 on device.
    """
    Wx = (W_ih @ W_in).astype(np.float32)
    bias = (W_ih @ b_in + b_ih + b_hh).astype(np.float32)
    w_gx = np.zeros((XPACK, 128, 4, 128), np.float32)
    w_gh = np.zeros((128, 4, 128), np.float32)
    for b in range(4):
        g = BANK_GATE[b]
        sc = 2.0 if b == 3 else 1.0
        for s in range(16):
            for r in range(H):
                col = 8 * s + r
                for tau in range(XPACK):
                    for k in range(IN):
                        w_gx[tau, 8 * s + 2 * tau + k, b, col] = sc * Wx[g * H + r, k]
                    w_gx[tau, 8 * s + 6, b, col] = sc * bias[g * H + r]
                w_gh[8 * s: 8 * s + H, b, col] = sc * W_hh[g * H + r, :]
    return w_gx.astype(np.float16), w_gh.astype(np.float16)


def _shuffle_state(aT):
    """[8, B_c] -> [128, N_PASS, N] device layout (8s+r, pass, n)."""
    return np.ascontiguousarray(
        aT.reshape(H, N_PASS, SLICES, N).transpose(2, 0, 1, 3).reshape(
            128, N_PASS, N).astype(np.float16))


def _unshuffle_state(dev):
    """[128, N_PASS, N] -> [8, B_c]."""
    return dev.reshape(SLICES, H, N_PASS, N).transpose(1, 2, 0, 3).reshape(
        H, B_C)


def _pack_x(x):
    """[T, 2, B_c] -> [n_chunk, N_PASS, 128, N]: 3 steps + ones row baked."""
    T = x.shape[0]
    n_chunk = (T + XPACK - 1) // XPACK
    out = np.zeros((n_chunk, N_PASS, SLICES, 8, N), np.float32)
    out[:, :, :, 6, :] = 1.0
    for tau in range(XPACK):
        for k in range(IN):
            for t3 in range(n_chunk):
                t = t3 * XPACK + tau
                if t < T:
                    out[t3, :, :, 2 * tau + k, :] = x[t, k].reshape(
                        N_PASS, SLICES, N)
    return np.ascontiguousarray(
        out.transpose(0, 2, 3, 1, 4).reshape(n_chunk, 128, N_PASS, N).astype(
            np.float16))


def _prep_core_inputs(inputs, lo, hi, weights):
    g = lambda k: np.asarray(inputs[k], np.float32)
    d = {}
    d["x_obs"] = _pack_x(
        np.ascontiguousarray(g("obs_traj_rel")[:, lo:hi, :].transpose(0, 2, 1)))
    d["x_pre"] = _pack_x(
        np.ascontiguousarray(g("pre_traj_rel")[:, lo:hi, :].transpose(0, 2, 1)))
    d["hT0"] = _shuffle_state(np.ascontiguousarray(g("h0")[lo:hi].T))
    d["cT0"] = _shuffle_state(np.ascontiguousarray(g("c0")[lo:hi].T))
    d["cT0_pre"] = _shuffle_state(np.ascontiguousarray(g("c0_pre")[lo:hi].T))
    d.update(weights)
    return d


# ------------------------------------------------------------- device build

def _build_kernel(tc, outs, ins):
    nc = tc.nc
    state = tc.alloc_tile_pool(name="state", bufs=1)
    psump = tc.alloc_tile_pool(name="psum", bufs=2, space="PSUM")

    wsb = {}
    for key in ("w_gx_obs", "w_gx_pre"):
        wsb[key] = state.tile([128, XPACK, 4, 128], F16, name=key + "_sb",
                              tag=key)
    for key in ("w_gh_obs", "w_gh_pre"):
        wsb[key] = state.tile([128, 4, 128], F16, name=key + "_sb", tag=key)

    def dma_weights(phase):
        nc.sync.dma_start(wsb[f"w_gx_{phase}"],
                          ins[f"w_gx_{phase}"].rearrange("t p b m -> p t b m"))
        nc.sync.dma_start(wsb[f"w_gh_{phase}"], ins[f"w_gh_{phase}"])

    # group tiles (4 chains each): h, c, tanh(c); x double-buffered for all 8
    n_grp = N_CHAINS // GRP
    hgrp = [state.tile([128, GRP, N], F16, name=f"hg_{g}", tag=f"hg_{g}")
            for g in range(n_grp)]
    cgrp = [state.tile([128, GRP, N], F16, name=f"cg_{g}", tag=f"cg_{g}")
            for g in range(n_grp)]
    tgrp = [state.tile([128, GRP, N], F16, name=f"tg_{g}", tag=f"tg_{g}")
            for g in range(n_grp)]
    xall = [state.tile([128, N_CHAINS, N], F16, name=f"xa_{i}", tag=f"xa_{i}")
            for i in range(2)]

    chains = []
    for ci in range(N_CHAINS):
        ch = {}
        for nm in ("gbar", "u", "v"):
            ch[nm] = state.tile([128, N], F16, name=f"{nm}_{ci}",
                                tag=f"{nm}_{ci}")
        ch["T"] = [
            state.tile([128, 4, N], F16, name=f"T_{ci}_{pb}", tag=f"T_{ci}_{pb}")
            for pb in range(2)
        ]
        ch["h"] = hgrp[ci // GRP][:, ci % GRP, :]
        ch["c"] = cgrp[ci // GRP][:, ci % GRP, :]
        ch["tc"] = tgrp[ci // GRP][:, ci % GRP, :]
        chains.append(ch)

    # global x-chunk schedule: (which, chunk idx, first step)
    chunk_seq = [("obs", k, k * XPACK) for k in range(N_CHUNK_OBS)] + \
                [("pre", k, T_OBS + k * XPACK) for k in range(N_CHUNK_PRE)]
    start_to_chunk = {st: i for i, (_, _, st) in enumerate(chunk_seq)}

    def dma_chunk(gi):
        which, k, _ = chunk_seq[gi]
        nc.sync.dma_start(xall[gi % 2], ins[f"x_{which}"][k])

    def dma_chunk_half(gi, g):
        which, k, _ = chunk_seq[gi]
        sl = slice(g * GRP, (g + 1) * GRP)
        nc.sync.dma_start(xall[gi % 2][:, sl, :],
                          ins[f"x_{which}"][k][:, sl, :])

    def mm_block(ch, wgx, wgh, tau, xt, pb):
        ps = psump.tile([128, 4, 512], F32, name="ps", tag="ps")
        for b in range(4):
            out = ps[:, b, :N]
            nc.tensor.matmul(out, wgx[:, tau, b, :], xt,
                             start=True, stop=False)
            nc.tensor.matmul(out, wgh[:, b, :], ch["h"],
                             start=False, stop=True)
        # single Sigmoid over all 4 banks (G pre-scaled by 2 in weights)
        T = ch["T"][pb]
        nc.scalar.activation(T[:, :, :], ps[:, :, :], AF.Sigmoid)
        # tg = 2*S_G - 1 == tanh(g)   (tensor_scalar, 4x mode)
        nc.vector.tensor_scalar(ch["gbar"], T[:, 3, :], 2.0, -1.0,
                                ALU.mult, ALU.add)
        nc.vector.tensor_mul(ch["u"], T[:, 0, :], ch["c"])   # F*c
        nc.vector.tensor_mul(ch["v"], T[:, 1, :], ch["gbar"])  # I*tanh(g)
        nc.vector.tensor_add(ch["c"], ch["u"], ch["v"])      # c' = u+v

    def group_tanh(g, pb):
        # one ACTIVATE for the whole group's c, then h = O * tanh(c)
        nc.scalar.activation(tgrp[g][:, :, :], cgrp[g][:, :, :], AF.Tanh)
        for ci in range(g * GRP, (g + 1) * GRP):
            ch = chains[ci]
            nc.vector.tensor_mul(ch["h"], ch["T"][pb][:, 2, :], ch["tc"])

    def grp_slice(ap, g):
        return ap[:, g * GRP:(g + 1) * GRP, :]

    assert N_PASS == N_CHAINS
    for t in range(T_OBS + T_PRE):
        if t < T_OBS:
            which, tt = "obs", t
        else:
            which, tt = "pre", t - T_OBS
        wgx, wgh = wsb[f"w_gx_{which}"], wsb[f"w_gh_{which}"]
        t3, tau = divmod(tt, XPACK)
        gi = start_to_chunk[t - tau]
        for ci in range(N_CHAINS):
            if ci == 2 and t > 0:
                group_tanh(1, (t - 1) % 2)  # tanh(c), chains 4-7, step t-1
                if t == T_OBS:
                    # h after obs phase (chains 4-7) + cell re-init
                    nc.sync.dma_start(grp_slice(outs["hT_obs"], 1), hgrp[1])
                    nc.sync.dma_start(cgrp[1], grp_slice(ins["cT0_pre"], 1))
            if ci == 0:
                if t == 0:
                    dma_weights("obs")
                    which0 = chunk_seq[0][0]
                    for j in range(GRP):
                        nc.sync.dma_start(xall[0][:, j, :],
                                          ins[f"x_{which0}"][0][:, j, :])
                        nc.sync.dma_start(hgrp[0][:, j, :],
                                          ins["hT0"][:, j, :])
                    nc.sync.dma_start(cgrp[0], grp_slice(ins["cT0"], 0))
                    dma_chunk_half(0, 1)
                    nc.sync.dma_start(hgrp[1], grp_slice(ins["hT0"], 1))
                    nc.sync.dma_start(cgrp[1], grp_slice(ins["cT0"], 1))
                if t == 1:
                    dma_weights("pre")
                    dma_chunk(1)  # prefetch chunk 1 (needed at t=3)
                if t == T_OBS:
                    nc.sync.dma_start(grp_slice(outs["hT_obs"], 0), hgrp[0])
                    nc.sync.dma_start(cgrp[0], grp_slice(ins["cT0_pre"], 0))
                if tau == 0 and t > 1 and gi + 1 < len(chunk_seq):
                    dma_chunk(gi + 1)  # prefetch next chunk
            mm_block(chains[ci], wgx, wgh, tau, xall[gi % 2][:, ci, :],
                     t % 2)
            if ci == GRP:
                group_tanh(0, t % 2)  # tanh(c) for chains 0-3 of step t
                if t == T_OBS + T_PRE - 1:
                    nc.sync.dma_start(grp_slice(outs["hT_pre"], 0), hgrp[0])
    # final group: per-chain h + immediate per-chain store to shorten the tail
    pb = (T_OBS + T_PRE - 1) % 2
    nc.scalar.activation(tgrp[1][:, :, :], cgrp[1][:, :, :], AF.Tanh)
    for ci in range(GRP, 2 * GRP):
        ch = chains[ci]
        nc.vector.tensor_mul(ch["h"], ch["T"][pb][:, 2, :], ch["tc"])
        nc.sync.dma_start(outs["hT_pre"][:, ci:ci + 1, :],
                          hgrp[1][:, ci - GRP:ci - GRP + 1, :])

    state.release()
    psump.release()


_CACHED = {}


def _get_program():
    if "nc" in _CACHED:
        return _CACHED["nc"], _CACHED["names"]
    nc = bacc.Bacc("TRN2", target_bir_lowering=False, debug=False,
                   enable_asserts=False, num_devices=N_CORES)
    in_shapes = {
        "x_obs": (N_CHUNK_OBS, 128, N_PASS, N),
        "x_pre": (N_CHUNK_PRE, 128, N_PASS, N),
        "hT0": (128, N_PASS, N),
        "cT0": (128, N_PASS, N),
        "cT0_pre": (128, N_PASS, N),
        "w_gx_obs": (XPACK, 128, 4, 128),
        "w_gh_obs": (128, 4, 128),
        "w_gx_pre": (XPACK, 128, 4, 128),
        "w_gh_pre": (128, 4, 128),
    }
    ins = {
        k: nc.dram_tensor(k, list(s), F16, kind="ExternalInput").ap()
        for k, s in in_shapes.items()
    }
    outs = {
        k: nc.dram_tensor(k, [128, N_PASS, N], F16, kind="ExternalOutput").ap()
        for k in ("hT_obs", "hT_pre")
    }
    with tile.TileContext(nc) as tc:
        _build_kernel(tc, outs, ins)
    nc.compile()
    _CACHED["nc"] = nc
    _CACHED["names"] = list(in_shapes)
    return nc, _CACHED["names"]


def run(inputs, trace=False, trace_kwargs=None):
    """Run the kernel on 8 cores; returns ((c_out, x_out), BassKernelResults)."""
    nc, _ = _get_program()
    g = lambda k: np.asarray(inputs[k], np.float32)
    wgx_o, wgh_o = _make_weights(g("W_in"), g("b_in"), g("W_ih_obs"),
                                 g("W_hh_obs"), g("b_ih_obs"), g("b_hh_obs"))
    wgx_p, wgh_p = _make_weights(g("W_in"), g("b_in"), g("W_ih_pre"),
                                 g("W_hh_pre"), g("b_ih_pre"), g("b_hh_pre"))
    weights = {"w_gx_obs": wgx_o, "w_gh_obs": wgh_o,
               "w_gx_pre": wgx_p, "w_gh_pre": wgh_p}
    in_maps = [
        _prep_core_inputs(inputs, c * B_C, (c + 1) * B_C, weights)
        for c in range(N_CORES)
    ]
    res = bass_utils.run_bass_kernel_spmd(
        nc, in_maps, core_ids=list(range(N_CORES)), trace=trace,
        **(trace_kwargs or {}))
    hT_obs = np.concatenate(
        [_unshuffle_state(res.results[c]["hT_obs"]) for c in range(N_CORES)],
        axis=1)
    hT_pre = np.concatenate(
        [_unshuffle_state(res.results[c]["hT_pre"]) for c in range(N_CORES)],
        axis=1)
    c_out = hT_obs.reshape(B, H).astype(np.float32)
    x_out = hT_pre.reshape(B, H).astype(np.float32)
    return (c_out, x_out), res


def kernel(**inputs):
    (c_out, x_out), _ = run(inputs)
    return c_out, x_out
